# revision 6
# baseline (speedup 1.0000x reference)
"""MinGRU kernel for Trainium2 (8 NeuronCores, Bass/Tile).

Reference computation (B=4, L=8192, D=512, fp32):
    gates = sigmoid(x @ Wg.T + bg)
    cands = tanh(x @ Wc.T + bc)
    h_t   = (1 - g_t) * h_{t-1} + g_t * c_t   (scan along L, h_0 = 0)

Sharding: core c -> (batch b = c//2, channel half = c%2). Each core computes
its batch's full L range for 256 of the 512 output channels; the scan along L
is per (b, channel) so no cross-core communication is needed.

Layout: host pre-transposes x[b] to [D, L] and weights to [D, 256] (lhsT) so
every device DMA is fully contiguous. On device, matmuls keep channels on
partitions and tokens on the free axis, which is exactly the layout
tensor_tensor_scan needs (recurrence runs along the free dim). The scan uses
    state = (a * state) - bneg,   a = sigmoid(-z_g - bg) = 1 - g,
    bneg = (a - 1) * c = -g * c
so a single scalar_tensor_tensor op feeds the scan. Output is [256, L] per
core; the host reassembles [B, L, D].
"""

import os
import sys

sys.path.insert(0, "/opt/trn_rl_repo")

import numpy as np

import concourse.bacc as bacc
import concourse.bass as bass
import concourse.mybir as mybir
from concourse.bass_utils import run_bass_kernel_spmd
from concourse.tile import TileContext

B, L, D = 4, 8192, 512
NCORES = 8
EH = D // 2          # output channels per core
NET = EH // 128      # e-tiles per core (2)
NDC = D // 128       # contraction chunks (4)
LT = 2048            # token tile (DMA granularity)
NSUB = 512           # matmul moving-operand free dim (fp32 max, = 1 PSUM bank)

FP32 = mybir.dt.float32
_last_results = None


def build_nc() -> bass.Bass:
    # Bacc (not plain Bass): its compile() runs move_matmul_waits_to_ldweights
    # and generate_event_semaphores, which split multi-sem waits to satisfy the
    # TRN2 per-instruction wait-slot limits walrus enforces.
    nc = bacc.Bacc()

    xT = nc.dram_tensor("xT", [D, L], FP32, kind="ExternalInput")
    wgT = nc.dram_tensor("wgT", [D, EH], FP32, kind="ExternalInput")
    wcT = nc.dram_tensor("wcT", [D, EH], FP32, kind="ExternalInput")
    bg = nc.dram_tensor("bg", [128, NET], FP32, kind="ExternalInput")
    bc = nc.dram_tensor("bc", [128, NET], FP32, kind="ExternalInput")
    h = nc.dram_tensor("h", [EH, L], FP32, kind="ExternalOutput")

    op = mybir.AluOpType
    act = mybir.ActivationFunctionType

    with TileContext(nc) as tc:
        with (
            tc.tile_pool(name="consts", bufs=1) as consts,
            tc.tile_pool(name="xpool", bufs=2) as xpool,
            tc.tile_pool(name="work", bufs=3) as work,
            tc.tile_pool(name="hpool", bufs=2) as hpool,
            tc.tile_pool(name="psum", bufs=2, space="PSUM") as psum,
        ):
            # Weights as [128, dc, e]: lhsT slices are [128, 128] contiguous.
            wg_sb = consts.tile([128, NDC, EH], FP32)
            wc_sb = consts.tile([128, NDC, EH], FP32)
            nc.sync.dma_start(wg_sb, wgT.rearrange("(c p) e -> p c e", p=128))
            nc.sync.dma_start(wc_sb, wcT.rearrange("(c p) e -> p c e", p=128))

            bg_raw = consts.tile([128, NET], FP32)
            bgn_sb = consts.tile([128, NET], FP32)
            bc_sb = consts.tile([128, NET], FP32)
            nc.sync.dma_start(bg_raw, bg[:])
            nc.sync.dma_start(bc_sb, bc[:])
            nc.scalar.mul(bgn_sb, bg_raw, -1.0)

            carry = [None] * NET  # [128, 1] AP of the previous h column

            for t in range(L // LT):
                x_sb = xpool.tile([128, NDC, LT], FP32, tag="x")
                nc.sync.dma_start(
                    x_sb, xT[:, t * LT : (t + 1) * LT].rearrange("(c p) l -> p c l", p=128)
                )
                h_sb = [
                    hpool.tile([128, LT], FP32, tag=f"h{et}", name=f"h{et}_{t}")
                    for et in range(NET)
                ]
                for n in range(LT // NSUB):
                    nsl = slice(n * NSUB, (n + 1) * NSUB)
                    for et in range(NET):
                        esl = slice(et * 128, (et + 1) * 128)
                        pg = psum.tile([128, NSUB], FP32, tag=f"pg{et}", name=f"pg{et}_{t}_{n}")
                        pc = psum.tile([128, NSUB], FP32, tag=f"pc{et}", name=f"pc{et}_{t}_{n}")
                        for dc in range(NDC):
                            nc.tensor.matmul(
                                pg,
                                wg_sb[:, dc, esl],
                                x_sb[:, dc, nsl],
                                start=(dc == 0),
                                stop=(dc == NDC - 1),
                            )
                        for dc in range(NDC):
                            nc.tensor.matmul(
                                pc,
                                wc_sb[:, dc, esl],
                                x_sb[:, dc, nsl],
                                start=(dc == 0),
                                stop=(dc == NDC - 1),
                            )
                        a_t = work.tile([128, NSUB], FP32, tag=f"a{et}", name=f"a{et}_{t}_{n}")
                        c_t = work.tile([128, NSUB], FP32, tag=f"c{et}", name=f"c{et}_{t}_{n}")
                        # a = sigmoid(-(z_g + bg)) = 1 - g
                        nc.scalar.activation(a_t, pg, act.Sigmoid, bias=bgn_sb[:, et : et + 1], scale=-1.0)
                        # c = tanh(z_c + bc)
                        nc.scalar.activation(c_t, pc, act.Tanh, bias=bc_sb[:, et : et + 1], scale=1.0)
                        # bneg = (a - 1) * c = -g * c
                        bn_t = work.tile([128, NSUB], FP32, tag=f"b{et}", name=f"b{et}_{t}_{n}")
                        nc.vector.scalar_tensor_tensor(bn_t, a_t, 1.0, c_t, op.subtract, op.mult)
                        # h = a * h_prev - bneg  (fp32 state in HW)
                        init = 0.0 if carry[et] is None else carry[et]
                        nc.vector.tensor_tensor_scan(
                            h_sb[et][:, nsl], a_t, bn_t, init, op.mult, op.subtract
                        )
                        carry[et] = h_sb[et][:, (n + 1) * NSUB - 1 : (n + 1) * NSUB]
                for et in range(NET):
                    nc.sync.dma_start(h[et * 128 : (et + 1) * 128, t * LT : (t + 1) * LT], h_sb[et])
    return nc


def _in_maps(x, Wg, bg, Wc, bc):
    maps = []
    xT = {}
    for c in range(NCORES):
        b, eh = c // 2, c % 2
        e0 = eh * EH
        if b not in xT:
            xT[b] = np.ascontiguousarray(x[b].T)
        maps.append(
            {
                "xT": xT[b],
                "wgT": np.ascontiguousarray(Wg[e0 : e0 + EH].T),
                "wcT": np.ascontiguousarray(Wc[e0 : e0 + EH].T),
                "bg": np.ascontiguousarray(bg[e0 : e0 + EH].reshape(NET, 128).T),
                "bc": np.ascontiguousarray(bc[e0 : e0 + EH].reshape(NET, 128).T),
            }
        )
    return maps


def kernel(x, Wg, bg, Wc, bc):
    global _last_results
    x = np.asarray(x, dtype=np.float32)
    Wg = np.asarray(Wg, dtype=np.float32)
    bg = np.asarray(bg, dtype=np.float32)
    Wc = np.asarray(Wc, dtype=np.float32)
    bc = np.asarray(bc, dtype=np.float32)

    nc = build_nc()
    if not nc.is_finalized():
        nc.finalize()
    res = run_bass_kernel_spmd(
        nc,
        _in_maps(x, Wg, bg, Wc, bc),
        list(range(NCORES)),
        tmpdir=os.environ.get("KERNEL_TMPDIR"),
    )
    _last_results = res

    out = np.empty((B, L, D), dtype=np.float32)
    for b in range(B):
        hb = np.concatenate([res.results[2 * b]["h"], res.results[2 * b + 1]["h"]], axis=0)
        out[b] = hb.T
    return out


# revision 9
# speedup vs baseline: 2.4185x; 2.4185x over previous
"""MinGRU kernel for Trainium2 (8 NeuronCores, Bass/Tile).

Reference computation (B=4, L=8192, D=512, fp32):
    gates = sigmoid(x @ Wg.T + bg)
    cands = tanh(x @ Wc.T + bc)
    h_t   = (1 - g_t) * h_{t-1} + g_t * c_t   (scan along L, h_0 = 0)

Sharding: core c -> (batch b = c//2, channel half = c%2). Each core computes
its batch's full L range for 256 of the 512 output channels; the scan along L
is per (b, channel) so no cross-core communication is needed.

Layout: host pre-transposes x[b] to [D, L] and weights to [D, 256] (lhsT) so
every device DMA is fully contiguous. On device, matmuls keep channels on
partitions and tokens on the free axis, which is exactly the layout
tensor_tensor_scan needs (recurrence runs along the free dim). The scan uses
    state = (a * state) - bneg,   a = sigmoid(-z_g - bg) = 1 - g,
    bneg = (a - 1) * c = -g * c
so a single scalar_tensor_tensor op feeds the scan. Output is [256, L] per
core; the host reassembles [B, L, D].
"""

import os
import sys

sys.path.insert(0, "/opt/trn_rl_repo")

import numpy as np

import concourse.bacc as bacc
import concourse.bass as bass
import concourse.mybir as mybir
from concourse.bass_utils import run_bass_kernel_spmd
from concourse.tile import TileContext

B, L, D = 4, 8192, 512
NCORES = 8
EH = D // 2          # output channels per core
NET = EH // 128      # e-tiles per core (2)
NDC = D // 128       # contraction chunks (4)
LT = 2048            # token tile (DMA granularity)
NSUB = 512           # matmul moving-operand free dim (fp32 max, = 1 PSUM bank)

FP32 = mybir.dt.float32
BF16 = mybir.dt.bfloat16
_last_results = None


def build_nc() -> bass.Bass:
    # Bacc (not plain Bass): its compile() runs move_matmul_waits_to_ldweights
    # and generate_event_semaphores, which split multi-sem waits to satisfy the
    # TRN2 per-instruction wait-slot limits walrus enforces.
    nc = bacc.Bacc()

    xT = nc.dram_tensor("xT", [D, L], FP32, kind="ExternalInput")
    wgT = nc.dram_tensor("wgT", [D, EH], FP32, kind="ExternalInput")
    wcT = nc.dram_tensor("wcT", [D, EH], FP32, kind="ExternalInput")
    bg = nc.dram_tensor("bg", [128, NET], FP32, kind="ExternalInput")
    bc = nc.dram_tensor("bc", [128, NET], FP32, kind="ExternalInput")
    h = nc.dram_tensor("h", [EH, L], FP32, kind="ExternalOutput")

    op = mybir.AluOpType
    act = mybir.ActivationFunctionType

    with TileContext(nc) as tc:
        with (
            tc.tile_pool(name="consts", bufs=1) as consts,
            tc.tile_pool(name="xpool", bufs=2) as xpool,
            tc.tile_pool(name="work", bufs=3) as work,
            tc.tile_pool(name="hpool", bufs=2) as hpool,
            tc.tile_pool(name="psum", bufs=2, space="PSUM") as psum,
        ):
            # Weights as [128, dc, e]: lhsT slices are [128, 128] contiguous.
            # bf16 in SBUF (gpsimd DMA casts fp32->bf16 inline): single-pass
            # matmuls instead of fp32 LOW_HIGH double-pumping.
            wg_sb = consts.tile([128, NDC, EH], BF16)
            wc_sb = consts.tile([128, NDC, EH], BF16)
            nc.gpsimd.dma_start(wg_sb, wgT.rearrange("(c p) e -> p c e", p=128))
            nc.gpsimd.dma_start(wc_sb, wcT.rearrange("(c p) e -> p c e", p=128))

            bg_raw = consts.tile([128, NET], FP32)
            bgn_sb = consts.tile([128, NET], FP32)
            bc_sb = consts.tile([128, NET], FP32)
            nc.sync.dma_start(bg_raw, bg[:])
            nc.sync.dma_start(bc_sb, bc[:])
            nc.scalar.mul(bgn_sb, bg_raw, -1.0)

            carry = [None] * NET  # [128, 1] AP of the previous h column

            for t in range(L // LT):
                x_sb = xpool.tile([128, NDC, LT], BF16, tag="x")
                nc.gpsimd.dma_start(
                    x_sb, xT[:, t * LT : (t + 1) * LT].rearrange("(c p) l -> p c l", p=128)
                )
                h_sb = [
                    hpool.tile([128, LT], FP32, tag=f"h{et}", name=f"h{et}_{t}")
                    for et in range(NET)
                ]
                for n in range(LT // NSUB):
                    nsl = slice(n * NSUB, (n + 1) * NSUB)
                    for et in range(NET):
                        esl = slice(et * 128, (et + 1) * 128)
                        pg = psum.tile([128, NSUB], FP32, tag=f"pg{et}", name=f"pg{et}_{t}_{n}")
                        pc = psum.tile([128, NSUB], FP32, tag=f"pc{et}", name=f"pc{et}_{t}_{n}")
                        for dc in range(NDC):
                            nc.tensor.matmul(
                                pg,
                                wg_sb[:, dc, esl],
                                x_sb[:, dc, nsl],
                                start=(dc == 0),
                                stop=(dc == NDC - 1),
                            )
                        for dc in range(NDC):
                            nc.tensor.matmul(
                                pc,
                                wc_sb[:, dc, esl],
                                x_sb[:, dc, nsl],
                                start=(dc == 0),
                                stop=(dc == NDC - 1),
                            )
                        a_t = work.tile([128, NSUB], FP32, tag=f"a{et}", name=f"a{et}_{t}_{n}")
                        c_t = work.tile([128, NSUB], FP32, tag=f"c{et}", name=f"c{et}_{t}_{n}")
                        # a = sigmoid(-(z_g + bg)) = 1 - g
                        nc.scalar.activation(a_t, pg, act.Sigmoid, bias=bgn_sb[:, et : et + 1], scale=-1.0)
                        # c = tanh(z_c + bc)
                        nc.scalar.activation(c_t, pc, act.Tanh, bias=bc_sb[:, et : et + 1], scale=1.0)
                        # bneg = (a - 1) * c = -g * c
                        bn_t = work.tile([128, NSUB], FP32, tag=f"b{et}", name=f"b{et}_{t}_{n}")
                        nc.vector.scalar_tensor_tensor(bn_t, a_t, 1.0, c_t, op.subtract, op.mult)
                        # h = a * h_prev - bneg  (fp32 state in HW)
                        init = 0.0 if carry[et] is None else carry[et]
                        nc.vector.tensor_tensor_scan(
                            h_sb[et][:, nsl], a_t, bn_t, init, op.mult, op.subtract
                        )
                        carry[et] = h_sb[et][:, (n + 1) * NSUB - 1 : (n + 1) * NSUB]
                for et in range(NET):
                    nc.sync.dma_start(h[et * 128 : (et + 1) * 128, t * LT : (t + 1) * LT], h_sb[et])
    return nc


def _in_maps(x, Wg, bg, Wc, bc):
    maps = []
    xT = {}
    for c in range(NCORES):
        b, eh = c // 2, c % 2
        e0 = eh * EH
        if b not in xT:
            xT[b] = np.ascontiguousarray(x[b].T)
        maps.append(
            {
                "xT": xT[b],
                "wgT": np.ascontiguousarray(Wg[e0 : e0 + EH].T),
                "wcT": np.ascontiguousarray(Wc[e0 : e0 + EH].T),
                "bg": np.ascontiguousarray(bg[e0 : e0 + EH].reshape(NET, 128).T),
                "bc": np.ascontiguousarray(bc[e0 : e0 + EH].reshape(NET, 128).T),
            }
        )
    return maps


def kernel(x, Wg, bg, Wc, bc):
    global _last_results
    x = np.asarray(x, dtype=np.float32)
    Wg = np.asarray(Wg, dtype=np.float32)
    bg = np.asarray(bg, dtype=np.float32)
    Wc = np.asarray(Wc, dtype=np.float32)
    bc = np.asarray(bc, dtype=np.float32)

    nc = build_nc()
    if not nc.is_finalized():
        nc.finalize()
    res = run_bass_kernel_spmd(
        nc,
        _in_maps(x, Wg, bg, Wc, bc),
        list(range(NCORES)),
        tmpdir=os.environ.get("KERNEL_TMPDIR"),
    )
    _last_results = res

    out = np.empty((B, L, D), dtype=np.float32)
    for b in range(B):
        hb = np.concatenate([res.results[2 * b]["h"], res.results[2 * b + 1]["h"]], axis=0)
        out[b] = hb.T
    return out


# revision 10
# speedup vs baseline: 2.4511x; 1.0135x over previous
"""MinGRU kernel for Trainium2 (8 NeuronCores, Bass/Tile).

Reference computation (B=4, L=8192, D=512, fp32):
    gates = sigmoid(x @ Wg.T + bg)
    cands = tanh(x @ Wc.T + bc)
    h_t   = (1 - g_t) * h_{t-1} + g_t * c_t   (scan along L, h_0 = 0)

Sharding: core c -> (batch b = c//2, channel half = c%2). Each core computes
its batch's full L range for 256 of the 512 output channels; the scan along L
is per (b, channel) so no cross-core communication is needed.

Layout: host pre-transposes x[b] to [D, L] and weights to [D, 256] (lhsT) so
every device DMA is fully contiguous. On device, matmuls keep channels on
partitions and tokens on the free axis, which is exactly the layout
tensor_tensor_scan needs (recurrence runs along the free dim). The scan uses
    state = (a * state) - bneg,   a = sigmoid(-z_g - bg) = 1 - g,
    bneg = (a - 1) * c = -g * c
so a single scalar_tensor_tensor op feeds the scan. Output is [256, L] per
core; the host reassembles [B, L, D].

Precision: x and W are cast fp32->fp16 on the way into SBUF (gpsimd DMA casts
inline; ACT casts the weights) so matmuls run single-pass on the PE instead of
fp32 LOW_HIGH double-pumping. PSUM accumulation and everything downstream
(activations, scan state) stay fp32.
"""

import os
import sys

sys.path.insert(0, "/opt/trn_rl_repo")

import numpy as np

import concourse.bacc as bacc
import concourse.bass as bass
import concourse.mybir as mybir
from concourse.bass_utils import run_bass_kernel_spmd
from concourse.tile import TileContext

B, L, D = 4, 8192, 512
NCORES = 8
EH = D // 2          # output channels per core
NET = EH // 128      # e-tiles per core (2)
NDC = D // 128       # contraction chunks (4)
LT = 1024            # token tile (DMA + scan granularity)
NSUB = 512           # matmul moving free dim (= 1 fp32 PSUM bank)
NNS = LT // NSUB     # matmul subtiles per token tile

FP32 = mybir.dt.float32
F16 = mybir.dt.float16
_last_results = None


def build_nc() -> bass.Bass:
    # Bacc (not plain Bass): its compile() runs move_matmul_waits_to_ldweights
    # and generate_event_semaphores, which split multi-sem waits to satisfy the
    # TRN2 per-instruction wait-slot limits walrus enforces.
    nc = bacc.Bacc()

    xT = nc.dram_tensor("xT", [D, L], FP32, kind="ExternalInput")
    wgT = nc.dram_tensor("wgT", [D, EH], FP32, kind="ExternalInput")
    wcT = nc.dram_tensor("wcT", [D, EH], FP32, kind="ExternalInput")
    bg = nc.dram_tensor("bg", [128, NET], FP32, kind="ExternalInput")
    bc = nc.dram_tensor("bc", [128, NET], FP32, kind="ExternalInput")
    h = nc.dram_tensor("h", [EH, L], FP32, kind="ExternalOutput")

    op = mybir.AluOpType
    act = mybir.ActivationFunctionType

    with TileContext(nc) as tc:
        with (
            tc.tile_pool(name="consts", bufs=1) as consts,
            tc.tile_pool(name="xpool", bufs=3) as xpool,
            tc.tile_pool(name="work", bufs=3) as work,
            tc.tile_pool(name="hpool", bufs=3) as hpool,
            tc.tile_pool(name="psum", bufs=2, space="PSUM") as psum,
        ):
            # Weights land as fp32 via HWDGE (keeps the SWDGE queue free for
            # the first x tile), then ACT casts them to fp16 once.
            wg_raw = consts.tile([128, NDC, EH], FP32)
            wc_raw = consts.tile([128, NDC, EH], FP32)
            nc.sync.dma_start(wg_raw, wgT.rearrange("(c p) e -> p c e", p=128))
            nc.sync.dma_start(wc_raw, wcT.rearrange("(c p) e -> p c e", p=128))
            wg_sb = consts.tile([128, NDC, EH], F16)
            wc_sb = consts.tile([128, NDC, EH], F16)
            nc.scalar.copy(wg_sb, wg_raw)
            nc.scalar.copy(wc_sb, wc_raw)

            bg_raw = consts.tile([128, NET], FP32)
            bgn_sb = consts.tile([128, NET], FP32)
            bc_sb = consts.tile([128, NET], FP32)
            nc.sync.dma_start(bg_raw, bg[:])
            nc.sync.dma_start(bc_sb, bc[:])
            nc.scalar.mul(bgn_sb, bg_raw, -1.0)

            carry = [None] * NET  # [128, 1] AP of the previous h column

            for t in range(L // LT):
                x_sb = xpool.tile([128, NDC, LT], F16, tag="x", name=f"x_{t}")
                nc.gpsimd.dma_start(
                    x_sb, xT[:, t * LT : (t + 1) * LT].rearrange("(c p) l -> p c l", p=128)
                )
                for et in range(NET):
                    esl = slice(et * 128, (et + 1) * 128)
                    a_t = work.tile([128, LT], FP32, tag=f"a{et}", name=f"a{et}_{t}")
                    c_t = work.tile([128, LT], FP32, tag=f"c{et}", name=f"c{et}_{t}")
                    for n in range(NNS):
                        nsl = slice(n * NSUB, (n + 1) * NSUB)
                        # One 2-bank PSUM tile per (g, c) pair: [*, 0, :] = z_g,
                        # [*, 1, :] = z_c.
                        pz = psum.tile(
                            [128, 2, NSUB], FP32, tag=f"pz{et}", name=f"pz{et}_{t}_{n}"
                        )
                        for dc in range(NDC):
                            nc.tensor.matmul(
                                pz[:, 0, :],
                                wg_sb[:, dc, esl],
                                x_sb[:, dc, nsl],
                                start=(dc == 0),
                                stop=(dc == NDC - 1),
                            )
                        for dc in range(NDC):
                            nc.tensor.matmul(
                                pz[:, 1, :],
                                wc_sb[:, dc, esl],
                                x_sb[:, dc, nsl],
                                start=(dc == 0),
                                stop=(dc == NDC - 1),
                            )
                        # a = sigmoid(-(z_g + bg)) = 1 - g ; c = tanh(z_c + bc)
                        nc.scalar.activation(
                            a_t[:, nsl], pz[:, 0, :], act.Sigmoid,
                            bias=bgn_sb[:, et : et + 1], scale=-1.0,
                        )
                        nc.scalar.activation(
                            c_t[:, nsl], pz[:, 1, :], act.Tanh,
                            bias=bc_sb[:, et : et + 1], scale=1.0,
                        )
                    # bneg = (a - 1) * c = -g * c  (one DVE op, full tile width)
                    bn_t = work.tile([128, LT], FP32, tag=f"b{et}", name=f"b{et}_{t}")
                    nc.vector.scalar_tensor_tensor(bn_t, a_t, 1.0, c_t, op.subtract, op.mult)
                    # h = a * h_prev - bneg  (fp32 state in HW)
                    h_t = hpool.tile([128, LT], FP32, tag=f"h{et}", name=f"h{et}_{t}")
                    init = 0.0 if carry[et] is None else carry[et]
                    nc.vector.tensor_tensor_scan(h_t, a_t, bn_t, init, op.mult, op.subtract)
                    carry[et] = h_t[:, LT - 1 : LT]
                    nc.sync.dma_start(h[et * 128 : (et + 1) * 128, t * LT : (t + 1) * LT], h_t)
    return nc


def _in_maps(x, Wg, bg, Wc, bc):
    maps = []
    xT = {}
    for c in range(NCORES):
        b, eh = c // 2, c % 2
        e0 = eh * EH
        if b not in xT:
            xT[b] = np.ascontiguousarray(x[b].T)
        maps.append(
            {
                "xT": xT[b],
                "wgT": np.ascontiguousarray(Wg[e0 : e0 + EH].T),
                "wcT": np.ascontiguousarray(Wc[e0 : e0 + EH].T),
                "bg": np.ascontiguousarray(bg[e0 : e0 + EH].reshape(NET, 128).T),
                "bc": np.ascontiguousarray(bc[e0 : e0 + EH].reshape(NET, 128).T),
            }
        )
    return maps


def kernel(x, Wg, bg, Wc, bc):
    global _last_results
    x = np.asarray(x, dtype=np.float32)
    Wg = np.asarray(Wg, dtype=np.float32)
    bg = np.asarray(bg, dtype=np.float32)
    Wc = np.asarray(Wc, dtype=np.float32)
    bc = np.asarray(bc, dtype=np.float32)

    nc = build_nc()
    if not nc.is_finalized():
        nc.finalize()
    res = run_bass_kernel_spmd(
        nc,
        _in_maps(x, Wg, bg, Wc, bc),
        list(range(NCORES)),
        tmpdir=os.environ.get("KERNEL_TMPDIR"),
    )
    _last_results = res

    out = np.empty((B, L, D), dtype=np.float32)
    for b in range(B):
        hb = np.concatenate([res.results[2 * b]["h"], res.results[2 * b + 1]["h"]], axis=0)
        out[b] = hb.T
    return out
